# revision 13
# baseline (speedup 1.0000x reference)
"""DopDense forward: relu(x @ (w * mult) + b) on 8 trn2 NeuronCores.

Swapped-operand layout vs the baseline: x^T tiles are the STATIONARY
matmul operand in fp8 e3m4 (measured: fp8 stationary with bf16 moving
sustains the full 216ns/512-col warm rate, unlike fp8 moving which is
~20% slower), and w is the resident MOVING operand in bf16.  This

- halves the x HBM traffic (8.4 -> 4.2 MB/core), which is what the
  contended multi-core lead-in and steady stream are bound by, and
- yields y in natural [batch, units] layout (psum [128 b, 512 u]), so
  the host unshard is a plain reshape.

The per-unit modulation `mult` lands on the free axis here, so it is
applied at EVICTION as a two-engine pipeline: ACT computes
relu(psum)->tmp (relu commutes with the positive per-unit scale) and
DVE multiplies by a broadcast [128,512] mult tile.  Both stages fit
under the 864ns/window matmul budget, and -- crucially -- the matmul
stream depends only on x and raw w, so the dop chain is never on the
PE critical path.  mult itself is built as mb = psL * psR where
psL/psR = 1 + scatter are each accumulated in PSUM by a rank-1 ones
matmul followed by a [128,128]x[128,512] mask matmul.

Power-of-two prescales keep fp8 in its normal range: x ships as 2x in
e3m4 and w as 0.5w in bf16 (exact); the dop columns ship as 32x in e3m4
with 1/32 folded into the host-side LOK10/ROK10 constants.

Schedule: ~52 FD=128 PE warm-up matmuls span the slow multi-core DMA
lead-in so the HAM clock gate is open when the real stream starts; x
arrives in 4-window 256KB groups (each dma_start costs ~600ns engine
time); one contiguous 128KB store per window round-robins over the
three rings, with gpsimd dropped near the end so its ~3.5us
end-of-context drain hides under the last windows' compute.

The ~7us semaphore-file reset storm appended by walrus codegen after
the final barrier is unconditional and counted in exec time.
"""

import numpy as np
import ml_dtypes


def _install_ntff_shim():
    """The trimmed antenv package in this image lacks axon_hooks, which
    concourse's trace=True path imports unconditionally.  Recreate the hook
    registry (and install the ctypes NTFF hook when available) so tracing
    works whether or not the caller enables it."""
    import sys
    import types
    try:
        import antenv
        import antenv.axon_hooks  # noqa: F401
        return
    except ImportError:
        pass
    try:
        import antenv
    except ImportError:
        return
    mod = types.ModuleType("antenv.axon_hooks")
    holder = [None]
    try:
        from trn_agent_boot.trn_boot import _ntff_profile_via_ctypes
        holder[0] = _ntff_profile_via_ctypes("/opt/axon/libaxon_pjrt.so")
    except Exception:
        pass
    mod.get_axon_ntff_profile_hook = lambda: holder[0]
    mod.set_axon_ntff_profile_hook = lambda h: holder.__setitem__(0, h)
    sys.modules["antenv.axon_hooks"] = mod
    antenv.axon_hooks = mod


_install_ntff_shim()

import concourse.bass as bass
import concourse.mybir as mybir
import concourse.tile as tile
from concourse import bacc
from concourse.bass_utils import run_bass_kernel_spmd

F32 = mybir.dt.float32
BF16 = mybir.dt.bfloat16
E3M4 = mybir.dt.float8e3
AF = mybir.ActivationFunctionType
ALU = mybir.AluOpType
BF16_NP = np.dtype(ml_dtypes.bfloat16)
E3M4_NP = np.dtype(ml_dtypes.float8_e3m4)

N_CORES = 8
B = 65536
NIN = 512
UNITS = 512
N_DOP = 128
SHARD = B // N_CORES          # 8192 batch rows per core
KC = NIN // 128               # 4 contraction chunks
NW = SHARD // 128             # 64 batch windows of 128 rows
GRP = 4                       # x windows per DMA group
NG = NW // GRP                # 16 x groups
THRESHOLD = 0.0
REF_PERIOD = 2.0
N_WARM = 36                   # FD=128 PE warm-up matmuls (~3.9us)
XS = np.float32(2.0)          # x prescale (folded out via w * 0.5)
DS = np.float32(32.0)         # dop-column prescale (folded into LOK10)

# Static dopaminergic-column index math (mirrors reference.py exactly)
DOP_IDX = np.linspace(1, UNITS - 1, N_DOP, dtype=np.int32)
LEFT_OK = ~np.isin(DOP_IDX - 1, DOP_IDX)
RIGHT_OK = ~np.isin(DOP_IDX + 1, DOP_IDX)
LCOL = (DOP_IDX - 1) % UNITS
RCOL = (DOP_IDX + 1) % UNITS

LOK10 = LEFT_OK.astype(np.float32) * np.float32(10.0 / NIN) / DS
ROK10 = RIGHT_OK.astype(np.float32) * np.float32(10.0 / NIN) / DS

_CACHED = {}


def build_nc(all_act: bool):
    if all_act in _CACHED:
        return _CACHED[all_act]
    nc = bacc.Bacc("TRN2", target_bir_lowering=False, debug=False,
                   num_swdge_queues=1)

    # x^T stationary tiles, fp8, 4-window groups:
    # xq[g][p, wi*512 + k*128 + j] = 2 * x[(4g+wi)*128 + j, k*128 + p]
    xq = nc.dram_tensor("xq", [NG, 128, GRP * KC * 128], E3M4,
                        kind="ExternalInput")
    # w moving chunks, pre-scaled by 0.5: wm[p, k*512+u] = 0.5*w[k*128+p, u]
    wm = nc.dram_tensor("wm", [128, KC * UNITS], BF16, kind="ExternalInput")
    # per-partition aux vectors: lok10/DS, rok10/DS, indicator, batch_ctr,
    # lcol (0..511), rcol (0..511)
    NV = 6
    auxs = nc.dram_tensor("auxs", [128, NV], F32, kind="ExternalInput")
    # dop columns of w^T and old^T, times 32, in fp8 e3m4
    auxb = nc.dram_tensor("auxb", [128, 2 * NIN], E3M4, kind="ExternalInput")
    # bias as a single-partition row (only read when b != 0)
    bline = nc.dram_tensor("bline", [1, UNITS], F32, kind="ExternalInput")
    # y windows in natural [batch, units] layout
    yo = nc.dram_tensor("yo", [NW, 128, UNITS], BF16, kind="ExternalOutput")

    with tile.TileContext(nc) as tc:
        with (
            tc.tile_pool(name="const", bufs=1) as const,
            tc.tile_pool(name="xg", bufs=4) as xpool,
            tc.tile_pool(name="ob", bufs=8) as opool,
        ):
            # warm-up scratch (also the zeros operand for the lf1/rf1
            # partition-broadcasts); PE dummies keep the HAM clock-gate
            # window busy through the contended multi-core DMA lead-in
            scr = const.tile([128, 128], BF16, tag="scr")
            nc.vector.memset(scr[:], 0.0)

            # ---------- input DMAs in consumption order ----------
            # axb rides the gpsimd ring: the dop chain only has to finish
            # by window 0's EVICTION (~+9us with psum slack), so sync's
            # critical prefix shrinks to axs + the first x half
            axs_sb = const.tile([128, NV], F32, tag="axs")
            nc.sync.dma_start(axs_sb[:], auxs[:])
            axb_sb = const.tile([128, 2 * NIN], E3M4, tag="axb")
            nc.gpsimd.dma_start(axb_sb[:], auxb[:])

            xg_tiles = {}

            def load_grp(g, eng):
                t = xpool.tile([128, GRP * KC * 128], E3M4, tag="xg",
                               name=f"xg{g}")
                xg_tiles[g] = t
                eng.dma_start(t[:], xq[g])

            # w ships in k-pieces so window 0's k-outer matmuls start after
            # only 128KB of w; x group 0 ships in 2-window halves likewise
            wm_sb = const.tile([128, KC * UNITS], BF16, tag="wm")
            for k in range(KC):
                nc.scalar.dma_start(wm_sb[:, k * UNITS:(k + 1) * UNITS],
                                    wm[:, k * UNITS:(k + 1) * UNITS])
            t0 = xpool.tile([128, GRP * KC * 128], E3M4, tag="xg",
                            name="xg0")
            xg_tiles[0] = t0
            half = GRP * KC * 128 // 2
            nc.sync.dma_start(t0[:, :half], xq[0][:, :half])
            nc.sync.dma_start(t0[:, half:], xq[0][:, half:])
            # spread the early groups over all three rings -- gpsimd is
            # otherwise idle during the bandwidth-starved lead-in
            load_grp(1, nc.scalar)
            load_grp(2, nc.gpsimd)
            load_grp(3, nc.sync)

            if all_act:
                bl_sb = const.tile([1, UNITS], F32, tag="bl")
                nc.sync.dma_start(bl_sb[:], bline[:])

            v_sb = axs_sb[:, 0:NV]
            wd_sb = axb_sb[:, 0:NIN]
            od_sb = axb_sb[:, NIN:2 * NIN]

            def x_tile(wp, k):
                g, wi = wp // GRP, wp % GRP
                return xg_tiles[g][:, wi * 512 + k * 128:
                                   wi * 512 + (k + 1) * 128]

            def wm_chunk(k):
                return wm_sb[:, k * UNITS:(k + 1) * UNITS]

            # iota over the full unit axis for the scatter row-masks
            io_sb = const.tile([128, UNITS], F32, tag="io")
            nc.gpsimd.iota(io_sb[:], [[1, UNITS]], base=0,
                           channel_multiplier=0,
                           allow_small_or_imprecise_dtypes=True)
            lmask = const.tile([128, UNITS], BF16, tag="lmask")
            nc.vector.tensor_scalar(lmask[:], io_sb[:], v_sb[:, 4:5],
                                    None, op0=ALU.is_equal)
            rmask = const.tile([128, UNITS], BF16, tag="rmask")
            nc.vector.tensor_scalar(rmask[:], io_sb[:], v_sb[:, 5:6],
                                    None, op0=ALU.is_equal)
            ones1 = const.tile([1, 128], BF16, tag="ones1")
            nc.vector.memset(ones1[:], 1.0)
            onesr = const.tile([1, UNITS], BF16, tag="onesr")
            nc.vector.memset(onesr[:], 1.0)

            # ---- dop chain: dd[j] = sum_i |32w - 32old|, gating, factors ----
            dch = const.tile([128, NIN], F32, tag="dch")
            nc.vector.tensor_tensor(dch[:], wd_sb, od_sb, op=ALU.subtract)
            dd = const.tile([128, 1], F32, tag="dd")
            nc.vector.tensor_reduce(
                dd[:], dch[:], axis=mybir.AxisListType.X, op=ALU.add,
                apply_absolute_value=True,
            )
            t1 = const.tile([128, 1], F32, tag="t1")
            nc.vector.tensor_tensor(t1[:], v_sb[:, 3:4], v_sb[:, 2:3],
                                    op=ALU.subtract)
            c2 = const.tile([128, 1], F32, tag="c2")
            nc.vector.tensor_scalar(c2[:], t1[:], REF_PERIOD, None,
                                    op0=ALU.is_gt)
            c1 = const.tile([128, 1], F32, tag="c1")
            nc.vector.tensor_scalar(c1[:], dd[:], THRESHOLD, None,
                                    op0=ALU.is_gt)
            av = const.tile([128, 1], F32, tag="av")
            nc.vector.tensor_tensor(av[:], c1[:], c2[:], op=ALU.mult)
            da = const.tile([128, 1], F32, tag="da")
            nc.vector.tensor_tensor(da[:], dd[:], av[:], op=ALU.mult)
            lf1 = const.tile([128, 1], F32, tag="lf1")
            nc.vector.tensor_tensor(lf1[:], da[:], v_sb[:, 0:1], op=ALU.mult)
            rf1 = const.tile([128, 1], F32, tag="rf1")
            nc.vector.tensor_tensor(rf1[:], da[:], v_sb[:, 1:2], op=ALU.mult)
            # partition-broadcasts for the scatter matmul stationaries:
            # lf1_rep[j, p] = lf1[j]  (scr is a zeros tile)
            lf1r = const.tile([128, 128], BF16, tag="lf1r")
            nc.vector.tensor_scalar(lf1r[:], scr[:], lf1[:], None,
                                    op0=ALU.add)
            rf1r = const.tile([128, 128], BF16, tag="rf1r")
            nc.vector.tensor_scalar(rf1r[:], scr[:], rf1[:], None,
                                    op0=ALU.add)

            with (
                tc.tile_pool(name="psx", bufs=2, space="PSUM") as psaux,
                tc.tile_pool(name="ps", bufs=6, space="PSUM") as pspool,
            ):
                warm = psaux.tile([128, 512], F32, tag="aux", name="warm")
                for _ in range(N_WARM):
                    nc.tensor.matmul(warm[:, :128], scr[:], scr[:],
                                     start=True, stop=True)

                # psL[p, u] = 1 + sum_j lf1[j] * lmask[j, u]   (all p equal)
                psL = psaux.tile([128, UNITS], F32, tag="aux", name="psL")
                nc.tensor.matmul(psL[:], ones1[:], onesr[:],
                                 start=True, stop=False)
                nc.tensor.matmul(psL[:], lf1r[:], lmask[:],
                                 start=False, stop=True)
                psR = psaux.tile([128, UNITS], F32, tag="aux", name="psR")
                nc.tensor.matmul(psR[:], ones1[:], onesr[:],
                                 start=True, stop=False)
                nc.tensor.matmul(psR[:], rf1r[:], rmask[:],
                                 start=False, stop=True)
                # DVE can read at most one PSUM input, so copy psL out first
                pls = const.tile([128, UNITS], F32, tag="pls")
                nc.vector.tensor_scalar(pls[:], psL[:], 0.0, None,
                                        op0=ALU.add)
                mb = const.tile([128, UNITS], BF16, tag="mb")
                nc.vector.tensor_tensor(mb[:], pls[:], psR[:], op=ALU.mult)
                if all_act:
                    mbf = const.tile([128, UNITS], F32, tag="mbf")
                    nc.vector.tensor_tensor(mbf[:], pls[:], psR[:],
                                            op=ALU.mult)
                    bbp = psaux.tile([128, UNITS], F32, tag="aux", name="bbp")
                    nc.tensor.matmul(bbp[:], ones1[:], bl_sb[:],
                                     start=True, stop=True)
                    bb_sb = const.tile([128, UNITS], F32, tag="bbs")
                    nc.vector.tensor_scalar(bb_sb[:], bbp[:], 0.0, None,
                                            op0=ALU.add)

                # ---------- main stream: y[b, u] windows ----------
                for wp in range(NW):
                    g = wp // GRP
                    if wp % GRP == 0 and 3 < g + 3 < NG:
                        eng = (nc.sync, nc.scalar, nc.gpsimd)[g % 3]
                        load_grp(g + 3, eng)
                    ps = pspool.tile([128, UNITS], F32, tag="mps", name="ps")
                    for k in range(KC):
                        nc.tensor.matmul(
                            ps[:], x_tile(wp, k), wm_chunk(k),
                            start=(k == 0), stop=(k == KC - 1))
                    ob = opool.tile([128, UNITS], BF16, tag="ob", name="ob")
                    if all_act:
                        # y = relu(z*mult + b): scale, add bias, relu
                        tmp = opool.tile([128, UNITS], F32, tag="tmp",
                                         name="tmp")
                        nc.vector.tensor_tensor(tmp[:], ps[:], mbf[:],
                                                op=ALU.mult)
                        tm2 = opool.tile([128, UNITS], F32, tag="tm2",
                                         name="tm2")
                        nc.vector.tensor_tensor(tm2[:], tmp[:], bb_sb[:],
                                                op=ALU.add)
                        nc.vector.tensor_scalar(ob[:], tm2[:], 0.0, None,
                                                op0=ALU.max)
                    else:
                        # per-unit mult on DVE straight from PSUM (bf16
                        # out), then relu alternating ACT/DVE -- relu
                        # commutes with the positive per-unit scale, and
                        # both engines stay well under the 864ns budget
                        tmp = opool.tile([128, UNITS], BF16, tag="tmp",
                                         name="tmp")
                        if wp == NW - 1:
                            # last window: both eviction stages in halves,
                            # pipelined across ACT/DVE, so the final half
                            # stores launch as early as possible
                            H = UNITS // 2
                            nc.vector.tensor_tensor(tmp[:, :H], ps[:, :H],
                                                    mb[:, :H], op=ALU.mult)
                            nc.scalar.activation(ob[:, :H], tmp[:, :H],
                                                 AF.Relu)
                            nc.vector.tensor_tensor(tmp[:, H:], ps[:, H:],
                                                    mb[:, H:], op=ALU.mult)
                            nc.vector.tensor_scalar(ob[:, H:], tmp[:, H:],
                                                    0.0, None, op0=ALU.max)
                        else:
                            nc.vector.tensor_tensor(tmp[:], ps[:], mb[:],
                                                    op=ALU.mult)
                            if wp % 2 == 0:
                                nc.scalar.activation(ob[:], tmp[:],
                                                     AF.Relu)
                            else:
                                nc.vector.tensor_scalar(ob[:], tmp[:], 0.0,
                                                        None, op0=ALU.max)
                    # one contiguous 128KB store per window; gpsimd drops
                    # out near the end so its end-of-context drain hides
                    if wp == NW - 1:
                        nc.sync.dma_start(yo[wp][:, :UNITS // 2],
                                          ob[:, :UNITS // 2])
                        nc.scalar.dma_start(yo[wp][:, UNITS // 2:],
                                            ob[:, UNITS // 2:])
                    elif wp > NW - 7:
                        eng = nc.sync if wp % 2 == 0 else nc.scalar
                        eng.dma_start(yo[wp][:], ob[:])
                    elif wp < 12:
                        # early windows: all stores on the gpsimd SWDGE ring
                        # (it backlogs ~2 windows at 120GB/s but catches up)
                        # so sync/scalar bandwidth stays on the x groups
                        nc.gpsimd.dma_start(yo[wp][:], ob[:])
                    else:
                        eng = (nc.gpsimd, nc.sync, nc.scalar)[wp % 3]
                        eng.dma_start(yo[wp][:], ob[:])

    nc.compile()
    _CACHED[all_act] = nc
    return nc


LAST_RESULTS = None


def kernel(x, w, b, dop_weights_old, indicator, batch_ctr):
    global LAST_RESULTS
    x = np.asarray(x, dtype=np.float32)
    w = np.ascontiguousarray(np.asarray(w, dtype=np.float32))
    b_arr = np.asarray(b, dtype=np.float32)
    old = np.asarray(dop_weights_old, dtype=np.float32)
    ind = np.asarray(indicator, dtype=np.float32)
    bc_val = float(np.asarray(batch_ctr).item())

    nc = build_nc(all_act=bool(np.any(b_arr)))

    # replicated inputs; reshapes/casts only -- the dop math runs on device
    wm = np.ascontiguousarray(
        (0.5 * w).reshape(KC, 128, UNITS).transpose(1, 0, 2)
    ).reshape(128, KC * UNITS).astype(BF16_NP)
    vcols = [LOK10, ROK10, ind.astype(np.float32),
             np.full(128, bc_val, np.float32),
             LCOL.astype(np.float32), RCOL.astype(np.float32)]
    auxs = np.ascontiguousarray(np.stack(vcols, axis=1).astype(np.float32))
    auxb = np.ascontiguousarray(np.concatenate(
        [w.T[DOP_IDX] * DS, old.T[DOP_IDX] * DS], axis=1, dtype=np.float32)
    ).astype(E3M4_NP)
    bl = np.ascontiguousarray(b_arr.reshape(1, UNITS))

    common = dict(wm=wm, auxs=auxs, auxb=auxb, bline=bl)

    xq_all = (XS * x).astype(E3M4_NP)
    in_maps = []
    for i in range(N_CORES):
        xs = xq_all[i * SHARD:(i + 1) * SHARD]       # [8192, 512] fp8
        # [g, wi, j, k, p] -> [g, p, wi, k, j]
        xqc = np.ascontiguousarray(
            xs.reshape(NG, GRP, 128, KC, 128).transpose(0, 4, 1, 3, 2)
        ).reshape(NG, 128, GRP * KC * 128)
        in_maps.append(dict(common, xq=xqc))

    res = run_bass_kernel_spmd(nc, in_maps, core_ids=list(range(N_CORES)))
    LAST_RESULTS = res

    out = np.empty((B, UNITS), np.float32)
    for i in range(N_CORES):
        out[i * SHARD:(i + 1) * SHARD] = (
            res.results[i]["yo"].reshape(SHARD, UNITS).astype(np.float32))
    return out
